# revision 11
# baseline (speedup 1.0000x reference)
"""Trainium2 Bass kernel: 7x7 valid 2D cross-correlation of an 8192x8192
fp32 image plus scalar bias, row-sharded across 8 NeuronCores.

Formulation (per core): the y-direction 7-tap convolution for a fixed kernel
column dx is a banded matmul: out_dx[y, x] = sum_r A_dx[r, y] * X[r, x] with
A_dx[r, y] = K[r - y, dx].  The full conv accumulates the 7 dx terms in PSUM
with the moving operand (image columns) shifted by dx.  Matmuls run in bf16
(inputs bf16, fp32 PSUM accumulate); the banded weight blocks are padded to
128 columns so the compiler's fast-weight-load path engages.

Sharding: image rows across the 8 cores; each core's input slab carries the
(kh-1)-row halo, so no on-device collectives are needed.
"""

import numpy as np
import ml_dtypes

import concourse.bass as bass
import concourse.mybir as mybir
from concourse.tile import TileContext
from concourse.bass_utils import run_bass_kernel_spmd

H = W = 8192
KH = KW = 7
OH = OW = H - KH + 1          # 8186
N_CORES = 8
OUT_ROWS = 1024               # per-core output rows (core 7: last 6 discarded)
IN_ROWS = OUT_ROWS + KH - 1   # 1030
BAND_IN = 128                 # input rows per matmul band (partition dim)
BAND_OUT = BAND_IN - KH + 1   # 122 output rows per band
APAD = 128                    # A block columns (padded from BAND_OUT for FWL)
COL_TILE = 512                # moving-operand free dim (one PSUM bank, fp32)
PS_GROUP = 4                  # column tiles per PSUM accumulation group
F32 = mybir.dt.float32
BF16 = mybir.dt.bfloat16

# Results object of the most recent hardware run (for test harnesses).
LAST_RESULTS = None


def _split_multi_waits(nc):
    """Walrus in this toolchain accepts at most ONE sync-wait per
    instruction; Tile's scheduler may attach several.  Hoist the extras onto
    single-wait InstEventSemaphore instructions inserted just before, on the
    same engine stream (a sequence of waits = AND of the conditions)."""
    uid = 0
    for fn in nc.m.functions:
        for blk in fn.blocks:
            new_list = []
            for inst in blk.instructions:
                si = getattr(inst, "sync_info", None)
                if si is not None and si.on_wait and len(si.on_wait) > 1:
                    waits = list(si.on_wait)
                    for w in waits[:-1]:
                        ev = mybir.InstEventSemaphore(
                            name=f"wait_split_{uid}",
                            ins=[],
                            outs=[],
                            sync_info=mybir.SyncInfo(on_wait=[w], on_update=[]),
                        )
                        uid += 1
                        ev.engine = inst.engine
                        new_list.append(ev)
                    si.on_wait = [waits[-1]]
                new_list.append(inst)
            blk.instructions[:] = new_list


def _band_starts(out_rows):
    starts = list(range(0, out_rows - BAND_OUT + 1, BAND_OUT))
    if starts[-1] + BAND_OUT < out_rows:
        starts.append(out_rows - BAND_OUT)   # final band overlaps its predecessor
    return starts


def _build_nc(bias_val, in_rows=IN_ROWS, out_rows=OUT_ROWS, w=W, ow=OW):
    nc = bass.Bass()
    X = nc.declare_dram_parameter("X", [in_rows, w], BF16, isOutput=False)
    A = nc.declare_dram_parameter("A", [BAND_IN, KW * APAD], BF16, isOutput=False)
    out = nc.declare_dram_parameter("out", [out_rows, ow], BF16, isOutput=True)

    starts = _band_starts(out_rows)
    col_starts = list(range(0, ow, COL_TILE))

    with TileContext(nc) as tc:
        with (
            tc.tile_pool(name="const", bufs=1) as cpool,
            tc.tile_pool(name="x", bufs=3) as xpool,
            tc.tile_pool(name="o", bufs=5) as opool,
            tc.tile_pool(name="ps", bufs=8, space="PSUM") as pspool,
        ):
            a_tile = cpool.tile([BAND_IN, KW * APAD], BF16)
            nc.scalar.dma_start(out=a_tile[:, :], in_=A[:, :])

            for s in starts:
                x_tile = xpool.tile([BAND_IN, w], BF16)
                # split loads/stores into partition chunks across the SWDGE
                # and HWDGE rings — a single big DMA drains through only a
                # few SDMA engines (descriptor-generation serialization)
                nc.gpsimd.dma_start(out=x_tile[0:64, :], in_=X[s : s + 64, :])
                nc.gpsimd.dma_start(out=x_tile[64:BAND_IN, :], in_=X[s + 64 : s + BAND_IN, :])
                o_tile = opool.tile([BAND_OUT, ow], BF16)

                # dx innermost: all 7 accumulating matmuls hit the same PSUM
                # bank back-to-back (fill/drain pipelining); DVE drains the
                # bank while the PE works on the next column tile.
                for x0 in col_starts:
                    n = min(COL_TILE, ow - x0)
                    ps = pspool.tile([APAD, COL_TILE], F32)
                    for dx in range(KW):
                        nc.tensor.matmul(
                            ps[:, :n],
                            lhsT=a_tile[:, dx * APAD : (dx + 1) * APAD],
                            rhs=x_tile[:, x0 + dx : x0 + dx + n],
                            start=(dx == 0),
                            stop=(dx == KW - 1),
                        )
                    nc.vector.tensor_scalar_add(
                        o_tile[:, x0 : x0 + n], ps[:BAND_OUT, :n], float(bias_val)
                    )

                # store the band (the overlapping final band stores only its
                # previously-unstored tail rows)
                prev_end = starts[starts.index(s) - 1] + BAND_OUT if starts.index(s) > 0 else 0
                lo = max(0, prev_end - s)
                nrows = BAND_OUT - lo
                nchunks = 8
                bounds = [lo + (nrows * k) // nchunks for k in range(nchunks + 1)]
                for k in range(nchunks):
                    p0, p1 = bounds[k], bounds[k + 1]
                    if p0 == p1:
                        continue
                    eng = (nc.gpsimd, nc.sync, nc.gpsimd, nc.scalar)[k % 4]
                    eng.dma_start(out=out[s + p0 : s + p1, :], in_=o_tile[p0:p1, :])
    _split_multi_waits(nc)
    return nc


def _make_A(K):
    A = np.zeros((BAND_IN, KW * APAD), np.float32)
    for dx in range(KW):
        for y in range(BAND_OUT):
            A[y : y + KH, dx * APAD + y] = K[:, dx]
    return A.astype(ml_dtypes.bfloat16)


def kernel(X, K, bias, _trace=False):
    global LAST_RESULTS
    X = np.asarray(X, dtype=np.float32)
    K = np.asarray(K, dtype=np.float32)
    bias_val = float(np.asarray(bias).reshape(-1)[0])

    A = _make_A(K)
    Xb = X.astype(ml_dtypes.bfloat16)
    # pad image rows so every core's slab has uniform [IN_ROWS, W] shape
    pad = N_CORES * OUT_ROWS + KH - 1 - H   # 6
    Xb = np.vstack([Xb, np.zeros((pad, W), ml_dtypes.bfloat16)])
    in_maps = [
        {"X": Xb[i * OUT_ROWS : i * OUT_ROWS + IN_ROWS], "A": A}
        for i in range(N_CORES)
    ]

    nc = _build_nc(bias_val)
    res = run_bass_kernel_spmd(nc, in_maps, core_ids=list(range(N_CORES)), trace=_trace)
    LAST_RESULTS = res

    full = np.concatenate(
        [res.results[i]["out"].astype(np.float32) for i in range(N_CORES)], axis=0
    )
    return np.ascontiguousarray(full[:OH])


# revision 13
# speedup vs baseline: 1.1928x; 1.1928x over previous
"""Trainium2 Bass kernel: 7x7 valid 2D cross-correlation of an 8192x8192
fp32 image plus scalar bias, row-sharded across 8 NeuronCores.

Formulation (per core): the y-direction 7-tap convolution for a fixed kernel
column dx is a banded matmul: out_dx[y, x] = sum_r A_dx[r, y] * X[r, x] with
A_dx[r, y] = K[r - y, dx].  The full conv accumulates the 7 dx terms in PSUM
with the moving operand (image columns) shifted by dx.  Matmuls run in bf16
(inputs bf16, fp32 PSUM accumulate); the banded weight blocks are padded to
128 columns so the compiler's fast-weight-load path engages.

Sharding: image rows across the 8 cores; each core's input slab carries the
(kh-1)-row halo, so no on-device collectives are needed.
"""

import numpy as np
import ml_dtypes

import concourse.bass as bass
import concourse.mybir as mybir
from concourse.tile import TileContext
from concourse.bass_utils import run_bass_kernel_spmd

H = W = 8192
KH = KW = 7
OH = OW = H - KH + 1          # 8186
N_CORES = 8
OUT_ROWS = 1024               # per-core output rows (core 7: last 6 discarded)
IN_ROWS = OUT_ROWS + KH - 1   # 1030
BAND_IN = 128                 # input rows per matmul band (partition dim)
BAND_OUT = BAND_IN - KH + 1   # 122 output rows per band
APAD = 128                    # A block columns (padded from BAND_OUT for FWL)
COL_TILE = 512                # moving-operand free dim (one PSUM bank, fp32)
PS_GROUP = 4                  # column tiles per PSUM accumulation group
F32 = mybir.dt.float32
BF16 = mybir.dt.bfloat16

# Results object of the most recent hardware run (for test harnesses).
LAST_RESULTS = None


def _split_multi_waits(nc):
    """Walrus in this toolchain accepts at most ONE sync-wait per
    instruction; Tile's scheduler may attach several.  Hoist the extras onto
    single-wait InstEventSemaphore instructions inserted just before, on the
    same engine stream (a sequence of waits = AND of the conditions)."""
    uid = 0
    for fn in nc.m.functions:
        for blk in fn.blocks:
            new_list = []
            for inst in blk.instructions:
                si = getattr(inst, "sync_info", None)
                if si is not None and si.on_wait and len(si.on_wait) > 1:
                    waits = list(si.on_wait)
                    for w in waits[:-1]:
                        ev = mybir.InstEventSemaphore(
                            name=f"wait_split_{uid}",
                            ins=[],
                            outs=[],
                            sync_info=mybir.SyncInfo(on_wait=[w], on_update=[]),
                        )
                        uid += 1
                        ev.engine = inst.engine
                        new_list.append(ev)
                    si.on_wait = [waits[-1]]
                new_list.append(inst)
            blk.instructions[:] = new_list


def _band_starts(out_rows):
    starts = list(range(0, out_rows - BAND_OUT + 1, BAND_OUT))
    if starts[-1] + BAND_OUT < out_rows:
        starts.append(out_rows - BAND_OUT)   # final band overlaps its predecessor
    return starts


def _build_nc(bias_val, in_rows=IN_ROWS, out_rows=OUT_ROWS, w=W, ow=OW):
    nc = bass.Bass()
    X = nc.declare_dram_parameter("X", [in_rows, w], BF16, isOutput=False)
    A = nc.declare_dram_parameter("A", [BAND_IN, KW * APAD], BF16, isOutput=False)
    out = nc.declare_dram_parameter("out", [out_rows, ow], BF16, isOutput=True)

    starts = _band_starts(out_rows)
    col_starts = list(range(0, ow, COL_TILE))

    with TileContext(nc) as tc:
        with (
            tc.tile_pool(name="const", bufs=1) as cpool,
            tc.tile_pool(name="x", bufs=4) as xpool,
            tc.tile_pool(name="o", bufs=5) as opool,
            tc.tile_pool(name="ps", bufs=8, space="PSUM") as pspool,
        ):
            a_tile = cpool.tile([BAND_IN, KW * APAD], BF16)
            nc.scalar.dma_start(out=a_tile[:, :], in_=A[:, :])

            # split loads/stores into partition chunks across the SWDGE and
            # HWDGE rings — a single big DMA drains through only a few SDMA
            # engines (descriptor-generation serialization).  Loads are issued
            # PREFETCH bands ahead of use so they never sit behind a store's
            # wait-for-DVE in the Pool queue (head-of-line blocking).
            PREFETCH = 3
            x_tiles = {}

            def issue_load(bi, first=False):
                if bi >= len(starts):
                    return
                sb = starts[bi]
                xt = xpool.tile([BAND_IN, w], BF16, tag="x")
                if first:
                    nc.gpsimd.dma_start(out=xt[0:32, :], in_=X[sb : sb + 32, :])
                    nc.gpsimd.dma_start(out=xt[32:64, :], in_=X[sb + 32 : sb + 64, :])
                    nc.sync.dma_start(out=xt[64:96, :], in_=X[sb + 64 : sb + 96, :])
                    nc.scalar.dma_start(out=xt[96:128, :], in_=X[sb + 96 : sb + 128, :])
                else:
                    nc.gpsimd.dma_start(out=xt[0:64, :], in_=X[sb : sb + 64, :])
                    nc.gpsimd.dma_start(out=xt[64:BAND_IN, :], in_=X[sb + 64 : sb + BAND_IN, :])
                x_tiles[bi] = xt

            for bi in range(PREFETCH):
                issue_load(bi, first=(bi == 0))

            for i, s in enumerate(starts):
                issue_load(i + PREFETCH)
                x_tile = x_tiles.pop(i)
                o_tile = opool.tile([BAND_OUT, ow], BF16)

                # dx innermost: all 7 accumulating matmuls hit the same PSUM
                # bank back-to-back (fill/drain pipelining); DVE drains the
                # bank while the PE works on the next column tile.
                for x0 in col_starts:
                    n = min(COL_TILE, ow - x0)
                    ps = pspool.tile([APAD, COL_TILE], F32)
                    for dx in range(KW):
                        nc.tensor.matmul(
                            ps[:, :n],
                            lhsT=a_tile[:, dx * APAD : (dx + 1) * APAD],
                            rhs=x_tile[:, x0 + dx : x0 + dx + n],
                            start=(dx == 0),
                            stop=(dx == KW - 1),
                        )
                    nc.vector.tensor_scalar_add(
                        o_tile[:, x0 : x0 + n], ps[:BAND_OUT, :n], float(bias_val)
                    )

                # store the band (the overlapping final band stores only its
                # previously-unstored tail rows)
                prev_end = starts[starts.index(s) - 1] + BAND_OUT if starts.index(s) > 0 else 0
                lo = max(0, prev_end - s)
                nrows = BAND_OUT - lo
                nchunks = 8
                bounds = [lo + (nrows * k) // nchunks for k in range(nchunks + 1)]
                for k in range(nchunks):
                    p0, p1 = bounds[k], bounds[k + 1]
                    if p0 == p1:
                        continue
                    eng = (nc.gpsimd, nc.sync, nc.gpsimd, nc.scalar)[k % 4]
                    eng.dma_start(out=out[s + p0 : s + p1, :], in_=o_tile[p0:p1, :])
    _split_multi_waits(nc)
    return nc


def _make_A(K):
    A = np.zeros((BAND_IN, KW * APAD), np.float32)
    for dx in range(KW):
        for y in range(BAND_OUT):
            A[y : y + KH, dx * APAD + y] = K[:, dx]
    return A.astype(ml_dtypes.bfloat16)


def kernel(X, K, bias, _trace=False):
    global LAST_RESULTS
    X = np.asarray(X, dtype=np.float32)
    K = np.asarray(K, dtype=np.float32)
    bias_val = float(np.asarray(bias).reshape(-1)[0])

    A = _make_A(K)
    Xb = X.astype(ml_dtypes.bfloat16)
    # pad image rows so every core's slab has uniform [IN_ROWS, W] shape
    pad = N_CORES * OUT_ROWS + KH - 1 - H   # 6
    Xb = np.vstack([Xb, np.zeros((pad, W), ml_dtypes.bfloat16)])
    in_maps = [
        {"X": Xb[i * OUT_ROWS : i * OUT_ROWS + IN_ROWS], "A": A}
        for i in range(N_CORES)
    ]

    nc = _build_nc(bias_val)
    res = run_bass_kernel_spmd(nc, in_maps, core_ids=list(range(N_CORES)), trace=_trace)
    LAST_RESULTS = res

    full = np.concatenate(
        [res.results[i]["out"].astype(np.float32) for i in range(N_CORES)], axis=0
    )
    return np.ascontiguousarray(full[:OH])
